# revision 16
# baseline (speedup 1.0000x reference)
"""Trainium2 Bass kernel for causal self-attention with cumulative-phase rotary
embedding (nn_CausalSelfAttention_64338610094602).

Sharding: 8 cores = 4 batches x 2 head-groups (tensor-parallel over heads).
Each core computes, for its (batch, 8-head group):
  omega/phi (replicated per batch), QKV projections, rotation + RMSNorm,
  causal attention (transposed-scores layout, max-free softmax), and a
  partial output projection. Host sums the two head-group partials per batch.

All big GEMMs run in float32r (full PE rate at N>=256, ~13-bit mantissa).
The phase/cumsum/trig path is kept in fp32.
"""
import math

import numpy as np
import ml_dtypes

import concourse.mybir as mybir
import concourse.tile as tile
from concourse import bacc
from concourse.bass_utils import run_bass_kernel_spmd

B, T, C = 4, 2048, 2048
H, D, DH = 16, 128, 64
HG = 8          # heads per core (head-group)
GD = HG * D     # group output dims = 1024
NT = T // 512   # 4 t-blocks of 512
NCT = C // 128  # 16 contraction tiles
EPS = 1e-5
SCL = 1.0 / math.sqrt(D)

dt = mybir.dt
AF = mybir.ActivationFunctionType
ALU = mybir.AluOpType

TWO_PI = 6.283185307179586
INV_2PI = 1.0 / TWO_PI
CW1 = float(np.float32(6.28125))
CW2 = float(np.float32(TWO_PI - 6.28125))
CW3 = float(TWO_PI - CW1 - float(np.float32(TWO_PI - 6.28125)))
MAGIC = 12582912.0  # 1.5 * 2^23: fp32 add/sub rounds to nearest int
HALF_PI = 1.5707963267948966
PI = 3.141592653589793

_CACHE = {}


def _round_f32r(x):
    """Round fp32 array to float32r (13-bit mantissa, round-to-nearest-even)."""
    x = np.ascontiguousarray(x, dtype=np.float32)
    b = x.view(np.uint32).copy()
    low = b & np.uint32(0x3FF)
    bb = b & ~np.uint32(0x3FF)
    rnd = (low > 0x200) | ((low == 0x200) & (((bb >> 10) & 1) == 1))
    return (bb + (rnd.astype(np.uint32) << 10)).view(np.float32)


def _build():
    f32, f32r, bf16 = dt.float32, dt.float32r, dt.bfloat16
    nc = bacc.Bacc(None, target_bir_lowering=False)
    with tile.TileContext(nc) as tc:
        xt_d = nc.dram_tensor("xt", (C, T), f32r, kind="ExternalInput")
        wq_d = nc.dram_tensor("wq", (C, GD), f32r, kind="ExternalInput")
        wk_d = nc.dram_tensor("wk", (C, GD), f32r, kind="ExternalInput")
        wv_d = nc.dram_tensor("wv", (C, GD), f32r, kind="ExternalInput")
        wo_d = nc.dram_tensor("wo", (GD, C), f32r, kind="ExternalInput")
        womg_d = nc.dram_tensor("womg", (128, NCT), f32r, kind="ExternalInput")
        b16_d = nc.dram_tensor("b16", (1, 1), f32, kind="ExternalInput")
        logf_d = nc.dram_tensor("logf", (DH, 1), f32, kind="ExternalInput")
        gq_d = nc.dram_tensor("gq", (128, 1), f32, kind="ExternalInput")
        gk_d = nc.dram_tensor("gk", (128, 1), f32, kind="ExternalInput")
        masks_d = nc.dram_tensor("masks", (128, 4 * 512), bf16, kind="ExternalInput")
        onesA_d = nc.dram_tensor("onesA", (128, 1), f32r, kind="ExternalInput")
        onesB_d = nc.dram_tensor("onesB", (1, 128), f32r, kind="ExternalInput")
        ones64_d = nc.dram_tensor("ones64", (1, DH), f32, kind="ExternalInput")
        oneh31_d = nc.dram_tensor("oneh31", (128, 31), f32r, kind="ExternalInput")
        out_d = nc.dram_tensor("out", (T, C), f32, kind="ExternalOutput")

        with tc.tile_pool(name="dram", bufs=1, space="DRAM") as dramp:
            yspill = dramp.tile([128, HG * T], f32r)  # yT per head at col h*T

            with tc.tile_pool(name="const", bufs=1) as constp:
                womg = constp.tile([128, NCT], f32r)
                nc.sync.dma_start(womg[:], womg_d[:])
                b16t = constp.tile([1, 1], f32)
                nc.sync.dma_start(b16t[:], b16_d[:])
                logf = constp.tile([DH, 1], f32)
                nc.sync.dma_start(logf[:], logf_d[:])
                gqt = constp.tile([128, 1], f32)
                nc.sync.dma_start(gqt[:], gq_d[:])
                gkt = constp.tile([128, 1], f32)
                nc.sync.dma_start(gkt[:], gk_d[:])
                onesA = constp.tile([128, 1], f32r)
                nc.sync.dma_start(onesA[:], onesA_d[:])
                onesB = constp.tile([1, 128], f32r)
                nc.sync.dma_start(onesB[:], onesB_d[:])
                ones64 = constp.tile([1, DH], f32)
                nc.sync.dma_start(ones64[:], ones64_d[:])
                epst = constp.tile([1, 1], f32)
                nc.vector.memset(epst[:], EPS)
                eps16 = constp.tile([16, 1], f32)
                nc.vector.memset(eps16[:], EPS)
                oneh31 = constp.tile([128, 31], f32r)
                nc.sync.dma_start(oneh31[:], oneh31_d[:])
                freq = constp.tile([DH, 1], f32)
                nc.scalar.activation(freq[:], logf[:], AF.Exp)

                _main(nc, tc, xt_d, wq_d, wk_d, wv_d, masks_d, yspill,
                      womg, b16t, gqt, gkt, onesA, onesB, ones64, eps16,
                      oneh31, freq)

                # ---- P3: output projection out = yall^T @ wo ----
                with tc.tile_pool(name="p3", bufs=1) as p3, \
                     tc.tile_pool(name="p3o", bufs=3) as p3o, \
                     tc.tile_pool(name="p3ps", bufs=4, space="PSUM") as p3ps:
                    yall = p3.tile([128, HG * T], f32r)
                    wosb = p3.tile([128, HG * C], f32r)
                    for h in range(HG):
                        nc.sync.dma_start(yall[:, h * T:(h + 1) * T],
                                          yspill[:, h * T:(h + 1) * T])
                        nc.sync.dma_start(wosb[:, h * C:(h + 1) * C],
                                          wo_d[h * 128:(h + 1) * 128, :])
                    for ti in range(T // 128):
                        for cb in range(C // 512):
                            ops = p3ps.tile([128, 512], f32, tag="o")
                            for h in range(HG):
                                nc.tensor.matmul(
                                    ops[:],
                                    yall[:, h * T + ti * 128:h * T + (ti + 1) * 128],
                                    wosb[:, h * C + cb * 512:h * C + (cb + 1) * 512],
                                    start=(h == 0), stop=(h == HG - 1))
                            osb = p3o.tile([128, 512], f32, tag="osb")
                            nc.vector.tensor_copy(osb[:], ops[:])
                            nc.sync.dma_start(
                                out_d[ti * 128:(ti + 1) * 128, cb * 512:(cb + 1) * 512],
                                osb[:])
    nc.compile()
    return nc


def _main(nc, tc, xt_d, wq_d, wk_d, wv_d, masks_d, yspill,
          womg, b16t, gqt, gkt, onesA, onesB, ones64, eps16, oneh31, freq):
    f32, f32r, bf16 = dt.float32, dt.float32r, dt.bfloat16

    with tc.tile_pool(name="big", bufs=1) as bigp, \
         tc.tile_pool(name="xtp", bufs=1) as xtp:
        trig = bigp.tile([128, T], f32)       # [0:64]=cos, [64:128]=sin
        masks = bigp.tile([128, 4 * 512], bf16)
        nc.sync.dma_start(masks[:], masks_d[:])

        xts = xtp.tile([128, NCT * T], f32r)  # c-tile i at cols [i*T, (i+1)*T)
        for i in range(NCT):
            nc.sync.dma_start(xts[:, i * T:(i + 1) * T],
                              xt_d[i * 128:(i + 1) * 128, :])

        # ---- P1: omega -> phi -> trig ----
        with tc.tile_pool(name="p1", bufs=1) as p1, \
             tc.tile_pool(name="p1b", bufs=2) as p1b, \
             tc.tile_pool(name="p1ps", bufs=2, space="PSUM") as p1ps:
            omega = p1.tile([1, T], f32)
            for J in range(NT):
                omps = p1ps.tile([1, 512], f32, tag="om")
                for i in range(NCT):
                    nc.tensor.matmul(
                        omps[:], womg[:, i:i + 1],
                        xts[:, i * T + J * 512:i * T + J * 512 + 512],
                        start=(i == 0), stop=(i == NCT - 1))
                nc.scalar.activation(omega[:, J * 512:(J + 1) * 512], omps[:],
                                     AF.Sigmoid, scale=1.0 / 16.0, bias=b16t[:])
            incl = p1.tile([1, T], f32)
            nc.vector.tensor_tensor_scan(incl[:], omega[:], omega[:], 0.0,
                                         ALU.add, ALU.bypass)
            phi = p1.tile([1, T], f32)
            nc.vector.tensor_sub(phi[:], incl[:], omega[:])
            for J in range(NT):
                sl = slice(J * 512, (J + 1) * 512)
                phps = p1ps.tile([DH, 512], f32, tag="phib")
                nc.tensor.matmul(phps[:], ones64[:], phi[:, sl],
                                 start=True, stop=True)
                ang = p1b.tile([DH, 512], f32, tag="ang")
                nc.vector.tensor_scalar(ang[:], phps[:], freq[:], None, op0=ALU.mult)
                mm = p1b.tile([DH, 512], f32, tag="mm")
                nc.vector.tensor_scalar(mm[:], ang[:], INV_2PI, MAGIC,
                                        op0=ALU.mult, op1=ALU.add)
                kk = p1b.tile([DH, 512], f32, tag="kk")
                nc.vector.tensor_scalar_add(kk[:], mm[:], -MAGIC)
                red = p1b.tile([DH, 512], f32, tag="red")
                nc.vector.cody_waite_cascade(red[:], ang[:], kk[:], CW1, CW2, CW3)
                red2 = p1b.tile([DH, 512], f32, tag="red2")
                nc.vector.add_range_wrap(red2[:], red[:], HALF_PI, PI, TWO_PI)
                nc.scalar.activation(trig[0:DH, sl], red2[:], AF.Sin)   # cos
                nc.scalar.activation(trig[DH:128, sl], red[:], AF.Sin)  # sin

        # ---- P2: per head-pair: QKV + rot/norm + attention ----
        with tc.tile_pool(name="qkv", bufs=1) as qkvp, \
             tc.tile_pool(name="wst", bufs=3) as wst, \
             tc.tile_pool(name="sc512", bufs=1) as sc512, \
             tc.tile_pool(name="rows", bufs=1) as rowsp:
            for pair in range(4):
                q_sb = qkvp.tile([128, 2 * T], f32r, tag="q", name=f"q_{pair}")
                k_sb = qkvp.tile([128, 2 * T], f32r, tag="k", name=f"k_{pair}")
                v_sb = qkvp.tile([128, 16 * 256], f32r, tag="v", name=f"v_{pair}")

                # --- 2a: q and k for both heads: matmul + rotate; rmsnorm is
                # batched per pair (one Ln + one Exp -> no ACT table thrash) ---
                with tc.tile_pool(name=f"ps2a_{pair}", bufs=1, space="PSUM") as psa, \
                     tc.tile_pool(name=f"ps2ax_{pair}", bufs=2, space="PSUM") as psax:
                    ssqps = psa.tile([16, 512], f32, tag="ssq",
                                     name=f"ssqps_{pair}")
                    site = 0
                    for wi, (w_d, gam, dest) in enumerate(
                            ((wq_d, gqt, q_sb), (wk_d, gkt, k_sb))):
                        for hl in range(2):
                            h = pair * 2 + hl
                            qps = [psa.tile([128, 512], f32, tag=f"qJ{J}",
                                            name=f"qp_{pair}_{wi}_{hl}_{J}")
                                   for J in range(NT)]
                            for i in range(NCT):
                                wt = wst.tile([128, 128], f32r, tag="w")
                                nc.sync.dma_start(
                                    wt[:],
                                    w_d[i * 128:(i + 1) * 128, h * 128:(h + 1) * 128])
                                for J in range(NT):
                                    nc.tensor.matmul(
                                        qps[J][:], wt[:],
                                        xts[:, i * T + J * 512:i * T + J * 512 + 512],
                                        start=(i == 0), stop=(i == NCT - 1))
                            for J in range(NT):
                                rot = _rotate(nc, sc512, qps[J], trig, J)
                                sq = sc512.tile([128, 512], f32r, tag="ta",
                                                name=f"sq_{pair}_{site}")
                                nc.scalar.activation(sq[:], rot[:], AF.Square)
                                nc.tensor.matmul(
                                    ssqps[:], oneh31[:, 15 - site:31 - site], sq[:],
                                    start=(site == 0), stop=(site == 15))
                                dcol = hl * T + J * 512
                                nc.scalar.copy(dest[:, dcol:dcol + 512], rot[:])
                                site += 1
                    # batched rstd = exp(-0.5 * ln(ssq/128 + eps)) for 16 sites
                    lnt = sc512.tile([16, 512], f32, tag="ta",
                                     name=f"lnt_{pair}")
                    nc.scalar.activation(lnt[:], ssqps[:], AF.Ln,
                                         scale=1.0 / 128.0, bias=eps16[:])
                    rstd = sc512.tile([16, 512], f32r, tag="tb",
                                      name=f"rstd_{pair}")
                    nc.scalar.activation(rstd[:], lnt[:], AF.Exp, scale=-0.5)
                    site = 0
                    for wi, (w_d, gam, dest) in enumerate(
                            ((wq_d, gqt, q_sb), (wk_d, gkt, k_sb))):
                        for hl in range(2):
                            for J in range(NT):
                                rrow = rowsp.tile([1, 512], f32r,
                                                  tag="r1" if site % 2 == 0 else "r0",
                                                  name=f"rrow_{pair}_{site}")
                                nc.sync.dma_start(rrow[:], rstd[site:site + 1, :])
                                rbps = psax.tile([128, 512], f32, tag="rb",
                                                 name=f"rb2a_{pair}_{site}")
                                nc.tensor.matmul(rbps[:], onesB[:], rrow[:],
                                                 start=True, stop=True)
                                dcol = hl * T + J * 512
                                nc.vector.scalar_tensor_tensor(
                                    dest[:, dcol:dcol + 512],
                                    dest[:, dcol:dcol + 512], gam[:], rbps[:],
                                    op0=ALU.mult, op1=ALU.mult)
                                site += 1

                # --- 2b: v for both heads (N=256 wide) ---
                with tc.tile_pool(name=f"ps2b_{pair}", bufs=1, space="PSUM") as psb:
                    for half in range(2):
                        vps = [psb.tile([128, 256], f32, tag=f"v{t}",
                                        name=f"vp_{pair}_{half}_{t}")
                               for t in range(8)]
                        for i in range(NCT):
                            wvt = wst.tile([128, 256], f32r, tag="wv")
                            nc.sync.dma_start(
                                wvt[:],
                                wv_d[i * 128:(i + 1) * 128,
                                     pair * 256:(pair + 1) * 256])
                            for t in range(8):
                                tt = half * 8 + t
                                nc.tensor.matmul(
                                    vps[t][:],
                                    xts[:, i * T + tt * 128:i * T + (tt + 1) * 128],
                                    wvt[:],
                                    start=(i == 0), stop=(i == NCT - 1))
                        for t in range(8):
                            tt = half * 8 + t
                            nc.vector.tensor_copy(
                                v_sb[:, tt * 256:(tt + 1) * 256], vps[t][:])

                # --- 2c: attention per head ---
                with tc.tile_pool(name=f"ps2c_{pair}", bufs=2, space="PSUM") as psc:
                    for hl in range(2):
                        h = pair * 2 + hl
                        for J in range(NT):
                            nI = 4 * J + 4
                            yps = psc.tile([128, 512], f32, tag="y",
                                           name=f"yps_{pair}_{hl}_{J}")
                            dps = psc.tile([1, 512], f32, tag="den", bufs=1,
                                           name=f"dps_{pair}_{hl}_{J}")
                            for I in range(nI):
                                sps = psc.tile([128, 512], f32, tag="s", bufs=3,
                                               name=f"sps_{pair}_{hl}_{J}_{I}")
                                nc.tensor.matmul(
                                    sps[:],
                                    k_sb[:, hl * T + I * 128:hl * T + (I + 1) * 128],
                                    q_sb[:, hl * T + J * 512:hl * T + (J + 1) * 512],
                                    start=True, stop=True)
                                ex = sc512.tile([128, 512], f32r,
                                                tag="ex" if I % 2 == 0 else "ex2",
                                                name=f"ex_{pair}_{hl}_{J}_{I}")
                                nc.scalar.activation(ex[:], sps[:], AF.Exp, scale=SCL)
                                if I >= 4 * J:
                                    r = I - 4 * J
                                    exm = sc512.tile([128, 512], f32r, tag="rot",
                                                     name=f"exm_{pair}_{hl}_{J}_{I}")
                                    nc.vector.tensor_tensor(
                                        exm[:], ex[:], masks[:, r * 512:(r + 1) * 512],
                                        op=ALU.mult)
                                    use = exm
                                else:
                                    use = ex
                                nc.tensor.matmul(
                                    yps[:],
                                    v_sb[:, I * 256 + hl * 128:I * 256 + hl * 128 + 128],
                                    use[:], start=(I == 0), stop=(I == nI - 1))
                                nc.tensor.matmul(
                                    dps[:], onesA[:], use[:],
                                    start=(I == 0), stop=(I == nI - 1))
                            rcf = rowsp.tile([1, 512], f32, tag="r0",
                                             name=f"rcf_{pair}_{hl}_{J}")
                            nc.vector.reciprocal_approx_fast(out=rcf[:], in_=dps[:])
                            recip = rowsp.tile([1, 512], f32r, tag="r1",
                                               name=f"recip_{pair}_{hl}_{J}")
                            nc.vector.tensor_copy(recip[:], rcf[:])
                            rbps = psc.tile([128, 512], f32, tag="rb",
                                            name=f"rbps_{pair}_{hl}_{J}")
                            nc.tensor.matmul(rbps[:], onesB[:], recip[:],
                                             start=True, stop=True)
                            rbsb = sc512.tile([128, 512], f32, tag="tb",
                                              name=f"rbsb_{pair}_{hl}_{J}")
                            nc.scalar.copy(rbsb[:], rbps[:])
                            yt = sc512.tile([128, 512], f32r, tag="ex2",
                                             name=f"yt_{pair}_{hl}_{J}")
                            nc.vector.tensor_tensor(yt[:], yps[:], rbsb[:],
                                                    op=ALU.mult)
                            nc.sync.dma_start(
                                yspill[:, h * T + J * 512:h * T + (J + 1) * 512],
                                yt[:])


def _rotate(nc, sc512, qps, trig, J):
    """Rotate (cumulative-phase RoPE) one (128, 512) projection PSUM tile.

    trig[0:64]=cos, [64:128]=sin for this J. Returns the rotated f32 tile.
    Ordered so the PSUM bank is released after the first 3 DVE ops."""
    f32 = dt.float32
    sl = slice(J * 512, (J + 1) * 512)
    ta = sc512.tile([DH, 512], f32, tag="ta")      # q1*cos
    tb = sc512.tile([DH, 512], f32, tag="tb")      # q2*sin
    tcc = sc512.tile([DH, 512], f32, tag="ex")     # q2*cos (ex slot: 2c-only)
    td = sc512.tile([DH, 512], f32, tag="ex2")     # q1*sin (ex2 slot: 2c-only)
    rot = sc512.tile([128, 512], f32, tag="rot")
    nc.vector.tensor_tensor(ta[:], qps[0:DH, :], trig[0:DH, sl], op=ALU.mult)
    nc.vector.tensor_tensor(tb[:], qps[DH:128, :], trig[DH:128, sl], op=ALU.mult)
    nc.vector.tensor_tensor(tcc[:], qps[DH:128, :], trig[0:DH, sl], op=ALU.mult)
    nc.vector.tensor_tensor(td[:], qps[0:DH, :], trig[DH:128, sl], op=ALU.mult)
    # PSUM bank free from here on
    nc.vector.tensor_add(rot[0:DH, :], ta[:], tb[:])
    nc.vector.tensor_sub(rot[DH:128, :], tcc[:], td[:])
    return rot


def _host_prep(inputs):
    x = np.asarray(inputs["x"], dtype=np.float32)
    Wq = np.asarray(inputs["Wq"], dtype=np.float32)
    Wk = np.asarray(inputs["Wk"], dtype=np.float32)
    Wv = np.asarray(inputs["Wv"], dtype=np.float32)
    Wo = np.asarray(inputs["Wo"], dtype=np.float32)
    w_omega = np.asarray(inputs["w_omega"], dtype=np.float32)
    b_omega = np.asarray(inputs["b_omega"], dtype=np.float32)
    log_freq = np.asarray(inputs["log_freq"], dtype=np.float32)
    q_gamma = np.asarray(inputs["q_gamma"], dtype=np.float32)
    k_gamma = np.asarray(inputs["k_gamma"], dtype=np.float32)

    womg = _round_f32r(w_omega.reshape(NCT, 128).T)  # [p, i] = w_omega[i*128+p]
    b16 = (b_omega / 16.0).reshape(1, 1).astype(np.float32)
    logf = log_freq.reshape(DH, 1)
    gq = q_gamma.reshape(128, 1)
    gk = k_gamma.reshape(128, 1)
    p = np.arange(128)[:, None]
    c = np.arange(512)[None, :]
    masks = np.concatenate(
        [((p + r * 128) <= c).astype(np.float32) for r in range(4)], axis=1
    ).astype(ml_dtypes.bfloat16)
    onesA = np.ones((128, 1), dtype=np.float32)
    onesB = np.ones((1, 128), dtype=np.float32)
    ones64 = np.ones((1, DH), dtype=np.float32)
    oneh31 = np.zeros((128, 31), dtype=np.float32)
    oneh31[:, 15] = 1.0

    in_maps = []
    for core in range(8):
        b, g = core // 2, core % 2
        in_maps.append({
            "xt": _round_f32r(x[b].T),
            "wq": _round_f32r(Wq[g * GD:(g + 1) * GD, :].T),
            "wk": _round_f32r(Wk[g * GD:(g + 1) * GD, :].T),
            "wv": _round_f32r(Wv[g * GD:(g + 1) * GD, :].T),
            "wo": _round_f32r(Wo[:, g * GD:(g + 1) * GD].T),
            "womg": womg, "b16": b16, "logf": logf, "gq": gq, "gk": gk,
            "masks": masks, "onesA": onesA, "onesB": onesB, "ones64": ones64,
            "oneh31": oneh31,
        })
    return in_maps


def kernel(**inputs) -> np.ndarray:
    if "nc" not in _CACHE:
        _CACHE["nc"] = _build()
    nc = _CACHE["nc"]
    in_maps = _host_prep(inputs)
    res = run_bass_kernel_spmd(nc, in_maps, core_ids=list(range(8)))
    out = np.empty((B, T, C), dtype=np.float32)
    for b in range(B):
        out[b] = res.results[2 * b]["out"] + res.results[2 * b + 1]["out"]
    return out


# revision 17
# speedup vs baseline: 1.1756x; 1.1756x over previous
"""Trainium2 Bass kernel for causal self-attention with cumulative-phase rotary
embedding (nn_CausalSelfAttention_64338610094602).

Sharding: 8 cores = 4 batches x 2 head-groups (tensor-parallel over heads).
Each core computes, for its (batch, 8-head group):
  omega/phi (replicated per batch), QKV projections, rotation + RMSNorm,
  causal attention (transposed-scores layout, max-free softmax), and a
  partial output projection. Host sums the two head-group partials per batch.

All big GEMMs run in float32r (full PE rate at N>=256, ~13-bit mantissa).
The phase/cumsum/trig path is kept in fp32.
"""
import math

import numpy as np
import ml_dtypes

import concourse.mybir as mybir
import concourse.tile as tile
from concourse import bacc
from concourse.bass_utils import run_bass_kernel_spmd

B, T, C = 4, 2048, 2048
H, D, DH = 16, 128, 64
HG = 8          # heads per core (head-group)
GD = HG * D     # group output dims = 1024
NT = T // 512   # 4 t-blocks of 512
NCT = C // 128  # 16 contraction tiles
EPS = 1e-5
SCL = 1.0 / math.sqrt(D)

dt = mybir.dt
AF = mybir.ActivationFunctionType
ALU = mybir.AluOpType

TWO_PI = 6.283185307179586
INV_2PI = 1.0 / TWO_PI
CW1 = float(np.float32(6.28125))
CW2 = float(np.float32(TWO_PI - 6.28125))
CW3 = float(TWO_PI - CW1 - float(np.float32(TWO_PI - 6.28125)))
MAGIC = 12582912.0  # 1.5 * 2^23: fp32 add/sub rounds to nearest int
HALF_PI = 1.5707963267948966
PI = 3.141592653589793

_CACHE = {}


def _round_f32r(x):
    """Round fp32 array to float32r (13-bit mantissa, round-to-nearest-even)."""
    x = np.ascontiguousarray(x, dtype=np.float32)
    b = x.view(np.uint32).copy()
    low = b & np.uint32(0x3FF)
    bb = b & ~np.uint32(0x3FF)
    rnd = (low > 0x200) | ((low == 0x200) & (((bb >> 10) & 1) == 1))
    return (bb + (rnd.astype(np.uint32) << 10)).view(np.float32)


def _build():
    f32, f32r, bf16 = dt.float32, dt.float32r, dt.bfloat16
    nc = bacc.Bacc(None, target_bir_lowering=False)
    with tile.TileContext(nc) as tc:
        xt_d = nc.dram_tensor("xt", (C, T), f32r, kind="ExternalInput")
        wq_d = nc.dram_tensor("wq", (C, GD), f32r, kind="ExternalInput")
        wk_d = nc.dram_tensor("wk", (C, GD), f32r, kind="ExternalInput")
        wv_d = nc.dram_tensor("wv", (C, GD), f32r, kind="ExternalInput")
        wo_d = nc.dram_tensor("wo", (GD, C), f32r, kind="ExternalInput")
        womg_d = nc.dram_tensor("womg", (128, NCT), f32r, kind="ExternalInput")
        b16_d = nc.dram_tensor("b16", (1, 1), f32, kind="ExternalInput")
        logf_d = nc.dram_tensor("logf", (DH, 1), f32, kind="ExternalInput")
        gq_d = nc.dram_tensor("gq", (128, 1), f32, kind="ExternalInput")
        gk_d = nc.dram_tensor("gk", (128, 1), f32, kind="ExternalInput")
        masks_d = nc.dram_tensor("masks", (128, 4 * 512), bf16, kind="ExternalInput")
        onesA_d = nc.dram_tensor("onesA", (128, 1), f32r, kind="ExternalInput")
        onesB_d = nc.dram_tensor("onesB", (1, 128), f32r, kind="ExternalInput")
        ones64_d = nc.dram_tensor("ones64", (1, DH), f32, kind="ExternalInput")
        oneh31_d = nc.dram_tensor("oneh31", (128, 31), f32r, kind="ExternalInput")
        out_d = nc.dram_tensor("out", (T, C), f32, kind="ExternalOutput")

        with tc.tile_pool(name="dram", bufs=1, space="DRAM") as dramp:
            yspill = dramp.tile([128, HG * T], f32r)  # yT per head at col h*T

            with tc.tile_pool(name="const", bufs=1) as constp:
                womg = constp.tile([128, NCT], f32r)
                nc.sync.dma_start(womg[:], womg_d[:])
                b16t = constp.tile([1, 1], f32)
                nc.sync.dma_start(b16t[:], b16_d[:])
                logf = constp.tile([DH, 1], f32)
                nc.sync.dma_start(logf[:], logf_d[:])
                gqt = constp.tile([128, 1], f32)
                nc.sync.dma_start(gqt[:], gq_d[:])
                gkt = constp.tile([128, 1], f32)
                nc.sync.dma_start(gkt[:], gk_d[:])
                onesA = constp.tile([128, 1], f32r)
                nc.sync.dma_start(onesA[:], onesA_d[:])
                onesB = constp.tile([1, 128], f32r)
                nc.sync.dma_start(onesB[:], onesB_d[:])
                ones64 = constp.tile([1, DH], f32)
                nc.sync.dma_start(ones64[:], ones64_d[:])
                epst = constp.tile([1, 1], f32)
                nc.vector.memset(epst[:], EPS)
                eps16 = constp.tile([16, 1], f32)
                nc.vector.memset(eps16[:], EPS)
                oneh31 = constp.tile([128, 31], f32r)
                nc.sync.dma_start(oneh31[:], oneh31_d[:])
                freq = constp.tile([DH, 1], f32)
                nc.scalar.activation(freq[:], logf[:], AF.Exp)

                _main(nc, tc, xt_d, wq_d, wk_d, wv_d, masks_d, yspill,
                      womg, b16t, gqt, gkt, onesA, onesB, ones64, eps16,
                      oneh31, freq)

                # ---- P3: output projection out = yall^T @ wo ----
                with tc.tile_pool(name="p3", bufs=1) as p3, \
                     tc.tile_pool(name="p3o", bufs=3) as p3o, \
                     tc.tile_pool(name="p3ps", bufs=4, space="PSUM") as p3ps:
                    yall = p3.tile([128, HG * T], f32r)
                    wosb = p3.tile([128, HG * C], f32r)
                    for h in range(HG):
                        nc.sync.dma_start(yall[:, h * T:(h + 1) * T],
                                          yspill[:, h * T:(h + 1) * T])
                        nc.sync.dma_start(wosb[:, h * C:(h + 1) * C],
                                          wo_d[h * 128:(h + 1) * 128, :])
                    for ti in range(T // 128):
                        for cb in range(C // 512):
                            ops = p3ps.tile([128, 512], f32, tag="o")
                            for h in range(HG):
                                nc.tensor.matmul(
                                    ops[:],
                                    yall[:, h * T + ti * 128:h * T + (ti + 1) * 128],
                                    wosb[:, h * C + cb * 512:h * C + (cb + 1) * 512],
                                    start=(h == 0), stop=(h == HG - 1))
                            osb = p3o.tile([128, 512], f32, tag="osb")
                            nc.vector.tensor_copy(osb[:], ops[:])
                            nc.sync.dma_start(
                                out_d[ti * 128:(ti + 1) * 128, cb * 512:(cb + 1) * 512],
                                osb[:])
    nc.compile()
    return nc


def _main(nc, tc, xt_d, wq_d, wk_d, wv_d, masks_d, yspill,
          womg, b16t, gqt, gkt, onesA, onesB, ones64, eps16, oneh31, freq):
    f32, f32r, bf16 = dt.float32, dt.float32r, dt.bfloat16

    with tc.tile_pool(name="big", bufs=1) as bigp, \
         tc.tile_pool(name="xtp", bufs=1) as xtp:
        trig = bigp.tile([128, T], f32)       # [0:64]=cos, [64:128]=sin
        masks = bigp.tile([128, 4 * 512], bf16)
        nc.sync.dma_start(masks[:], masks_d[:])

        xts = xtp.tile([128, NCT * T], f32r)  # c-tile i at cols [i*T, (i+1)*T)
        for i in range(NCT):
            nc.sync.dma_start(xts[:, i * T:(i + 1) * T],
                              xt_d[i * 128:(i + 1) * 128, :])

        # ---- P1: omega -> phi -> trig ----
        with tc.tile_pool(name="p1", bufs=1) as p1, \
             tc.tile_pool(name="p1b", bufs=2) as p1b, \
             tc.tile_pool(name="p1ps", bufs=2, space="PSUM") as p1ps:
            omega = p1.tile([1, T], f32)
            for J in range(NT):
                omps = p1ps.tile([1, 512], f32, tag="om")
                for i in range(NCT):
                    nc.tensor.matmul(
                        omps[:], womg[:, i:i + 1],
                        xts[:, i * T + J * 512:i * T + J * 512 + 512],
                        start=(i == 0), stop=(i == NCT - 1))
                nc.scalar.activation(omega[:, J * 512:(J + 1) * 512], omps[:],
                                     AF.Sigmoid, scale=1.0 / 16.0, bias=b16t[:])
            incl = p1.tile([1, T], f32)
            nc.vector.tensor_tensor_scan(incl[:], omega[:], omega[:], 0.0,
                                         ALU.add, ALU.bypass)
            phi = p1.tile([1, T], f32)
            nc.vector.tensor_sub(phi[:], incl[:], omega[:])
            for J in range(NT):
                sl = slice(J * 512, (J + 1) * 512)
                phps = p1ps.tile([DH, 512], f32, tag="phib")
                nc.tensor.matmul(phps[:], ones64[:], phi[:, sl],
                                 start=True, stop=True)
                ang = p1b.tile([DH, 512], f32, tag="ang")
                nc.vector.tensor_scalar(ang[:], phps[:], freq[:], None, op0=ALU.mult)
                mm = p1b.tile([DH, 512], f32, tag="mm")
                nc.vector.tensor_scalar(mm[:], ang[:], INV_2PI, MAGIC,
                                        op0=ALU.mult, op1=ALU.add)
                kk = p1b.tile([DH, 512], f32, tag="kk")
                nc.vector.tensor_scalar_add(kk[:], mm[:], -MAGIC)
                red = p1b.tile([DH, 512], f32, tag="red")
                nc.vector.cody_waite_cascade(red[:], ang[:], kk[:], CW1, CW2, CW3)
                red2 = p1b.tile([DH, 512], f32, tag="red2")
                nc.vector.add_range_wrap(red2[:], red[:], HALF_PI, PI, TWO_PI)
                nc.scalar.activation(trig[0:DH, sl], red2[:], AF.Sin)   # cos
                nc.scalar.activation(trig[DH:128, sl], red[:], AF.Sin)  # sin

        # ---- P2: per head-pair: QKV + rot/norm + attention ----
        with tc.tile_pool(name="qkv", bufs=1) as qkvp, \
             tc.tile_pool(name="wst", bufs=3) as wst, \
             tc.tile_pool(name="sc512", bufs=1) as sc512, \
             tc.tile_pool(name="rows", bufs=1) as rowsp:
            for pair in range(4):
                q_sb = qkvp.tile([128, 2 * T], f32r, tag="q", name=f"q_{pair}")
                k_sb = qkvp.tile([128, 2 * T], f32r, tag="k", name=f"k_{pair}")
                v_sb = qkvp.tile([128, 16 * 256], f32r, tag="v", name=f"v_{pair}")

                # --- 2a: q and k for both heads: matmul + rotate; rmsnorm is
                # batched per pair (one Ln + one Exp -> no ACT table thrash) ---
                with tc.tile_pool(name=f"ps2a_{pair}", bufs=1, space="PSUM") as psa, \
                     tc.tile_pool(name=f"ps2ax_{pair}", bufs=2, space="PSUM") as psax:
                    ssqps = psa.tile([16, 512], f32, tag="ssq",
                                     name=f"ssqps_{pair}")
                    site = 0
                    for wi, (w_d, gam, dest) in enumerate(
                            ((wq_d, gqt, q_sb), (wk_d, gkt, k_sb))):
                        for hl in range(2):
                            h = pair * 2 + hl
                            qps = [psa.tile([128, 512], f32, tag=f"qJ{J}",
                                            name=f"qp_{pair}_{wi}_{hl}_{J}")
                                   for J in range(NT)]
                            for i in range(NCT):
                                wt = wst.tile([128, 128], f32r, tag="w")
                                nc.sync.dma_start(
                                    wt[:],
                                    w_d[i * 128:(i + 1) * 128, h * 128:(h + 1) * 128])
                                for J in range(NT):
                                    nc.tensor.matmul(
                                        qps[J][:], wt[:],
                                        xts[:, i * T + J * 512:i * T + J * 512 + 512],
                                        start=(i == 0), stop=(i == NCT - 1))
                            for J in range(NT):
                                rot = _rotate(nc, sc512, qps[J], trig, J)
                                sq = sc512.tile([128, 512], f32r, tag="ta",
                                                name=f"sq_{pair}_{site}")
                                nc.scalar.activation(sq[:], rot[:], AF.Square)
                                nc.tensor.matmul(
                                    ssqps[:], oneh31[:, 15 - site:31 - site], sq[:],
                                    start=(site == 0), stop=(site == 15))
                                dcol = hl * T + J * 512
                                nc.scalar.copy(dest[:, dcol:dcol + 512], rot[:])
                                site += 1
                    # batched rstd = exp(-0.5 * ln(ssq/128 + eps)) for 16 sites
                    lnt = sc512.tile([16, 512], f32, tag="ta",
                                     name=f"lnt_{pair}")
                    nc.scalar.activation(lnt[:], ssqps[:], AF.Ln,
                                         scale=1.0 / 128.0, bias=eps16[:])
                    rstd = sc512.tile([16, 512], f32r, tag="tb",
                                      name=f"rstd_{pair}")
                    nc.scalar.activation(rstd[:], lnt[:], AF.Exp, scale=-0.5)
                    site = 0
                    for wi, (w_d, gam, dest) in enumerate(
                            ((wq_d, gqt, q_sb), (wk_d, gkt, k_sb))):
                        for hl in range(2):
                            for J in range(NT):
                                rrow = rowsp.tile([1, 512], f32r,
                                                  tag="r1" if site % 2 == 0 else "r0",
                                                  name=f"rrow_{pair}_{site}")
                                nc.sync.dma_start(rrow[:], rstd[site:site + 1, :])
                                rbps = psax.tile([128, 512], f32, tag="rb",
                                                 name=f"rb2a_{pair}_{site}")
                                nc.tensor.matmul(rbps[:], onesB[:], rrow[:],
                                                 start=True, stop=True)
                                dcol = hl * T + J * 512
                                nc.vector.scalar_tensor_tensor(
                                    dest[:, dcol:dcol + 512],
                                    dest[:, dcol:dcol + 512], gam[:], rbps[:],
                                    op0=ALU.mult, op1=ALU.mult)
                                site += 1

                # --- 2b: v for both heads (N=256 wide) ---
                with tc.tile_pool(name=f"ps2b_{pair}", bufs=1, space="PSUM") as psb:
                    for half in range(2):
                        vps = [psb.tile([128, 256], f32, tag=f"v{t}",
                                        name=f"vp_{pair}_{half}_{t}")
                               for t in range(8)]
                        for i in range(NCT):
                            wvt = wst.tile([128, 256], f32r, tag="wv")
                            nc.sync.dma_start(
                                wvt[:],
                                wv_d[i * 128:(i + 1) * 128,
                                     pair * 256:(pair + 1) * 256])
                            for t in range(8):
                                tt = half * 8 + t
                                nc.tensor.matmul(
                                    vps[t][:],
                                    xts[:, i * T + tt * 128:i * T + (tt + 1) * 128],
                                    wvt[:],
                                    start=(i == 0), stop=(i == NCT - 1))
                        for t in range(8):
                            tt = half * 8 + t
                            nc.vector.tensor_copy(
                                v_sb[:, tt * 256:(tt + 1) * 256], vps[t][:])

                # --- 2c: attention per head; each J's softmax epilogue is
                # emitted inside the next J's matmul stream so the in-order
                # PE queue never stalls on the recip -> broadcast chain ---
                with tc.tile_pool(name=f"ps2c_{pair}", bufs=2, space="PSUM") as psc:
                    pend = [None]

                    def epilogue(yps, dps, hl, J):
                        h = pair * 2 + hl
                        rcf = rowsp.tile([1, 512], f32, tag="r0",
                                         name=f"rcf_{pair}_{hl}_{J}")
                        nc.vector.reciprocal_approx_fast(out=rcf[:], in_=dps[:])
                        recip = rowsp.tile([1, 512], f32r, tag="r1",
                                           name=f"recip_{pair}_{hl}_{J}")
                        nc.vector.tensor_copy(recip[:], rcf[:])
                        rbps = psc.tile([128, 512], f32, tag="rb", bufs=1,
                                        name=f"rbps_{pair}_{hl}_{J}")
                        nc.tensor.matmul(rbps[:], onesB[:], recip[:],
                                         start=True, stop=True)
                        rbsb = sc512.tile([128, 512], f32, tag="tb",
                                          name=f"rbsb_{pair}_{hl}_{J}")
                        nc.scalar.copy(rbsb[:], rbps[:])
                        yt = sc512.tile([128, 512], f32r, tag="ta",
                                        name=f"yt_{pair}_{hl}_{J}")
                        nc.vector.tensor_tensor(yt[:], yps[:], rbsb[:],
                                                op=ALU.mult)
                        nc.sync.dma_start(
                            yspill[:, h * T + J * 512:h * T + (J + 1) * 512],
                            yt[:])

                    for hl in range(2):
                        for J in range(NT):
                            nI = 4 * J + 4
                            yps = psc.tile([128, 512], f32, tag="y",
                                           name=f"yps_{pair}_{hl}_{J}")
                            dps = psc.tile([1, 512], f32, tag="den",
                                           name=f"dps_{pair}_{hl}_{J}")
                            for I in range(nI):
                                sps = psc.tile([128, 512], f32, tag="s", bufs=3,
                                               name=f"sps_{pair}_{hl}_{J}_{I}")
                                nc.tensor.matmul(
                                    sps[:],
                                    k_sb[:, hl * T + I * 128:hl * T + (I + 1) * 128],
                                    q_sb[:, hl * T + J * 512:hl * T + (J + 1) * 512],
                                    start=True, stop=True)
                                ex = sc512.tile([128, 512], f32r,
                                                tag="ex" if I % 2 == 0 else "ex2",
                                                name=f"ex_{pair}_{hl}_{J}_{I}")
                                nc.scalar.activation(ex[:], sps[:], AF.Exp, scale=SCL)
                                if I >= 4 * J:
                                    r = I - 4 * J
                                    exm = sc512.tile([128, 512], f32r, tag="rot",
                                                     name=f"exm_{pair}_{hl}_{J}_{I}")
                                    nc.vector.tensor_tensor(
                                        exm[:], ex[:], masks[:, r * 512:(r + 1) * 512],
                                        op=ALU.mult)
                                    use = exm
                                else:
                                    use = ex
                                nc.tensor.matmul(
                                    yps[:],
                                    v_sb[:, I * 256 + hl * 128:I * 256 + hl * 128 + 128],
                                    use[:], start=(I == 0), stop=(I == nI - 1))
                                nc.tensor.matmul(
                                    dps[:], onesA[:], use[:],
                                    start=(I == 0), stop=(I == nI - 1))
                                if I == 1 and pend[0] is not None:
                                    pend[0]()
                                    pend[0] = None
                            pend[0] = (lambda yps=yps, dps=dps, hl=hl, J=J:
                                       epilogue(yps, dps, hl, J))
                    pend[0]()
                    pend[0] = None


def _rotate(nc, sc512, qps, trig, J):
    """Rotate (cumulative-phase RoPE) one (128, 512) projection PSUM tile.

    trig[0:64]=cos, [64:128]=sin for this J. Returns the rotated f32 tile.
    Ordered so the PSUM bank is released after the first 3 DVE ops."""
    f32 = dt.float32
    sl = slice(J * 512, (J + 1) * 512)
    ta = sc512.tile([DH, 512], f32, tag="ta")      # q1*cos
    tb = sc512.tile([DH, 512], f32, tag="tb")      # q2*sin
    tcc = sc512.tile([DH, 512], f32, tag="ex")     # q2*cos (ex slot: 2c-only)
    td = sc512.tile([DH, 512], f32, tag="ex2")     # q1*sin (ex2 slot: 2c-only)
    rot = sc512.tile([128, 512], f32, tag="rot")
    nc.vector.tensor_tensor(ta[:], qps[0:DH, :], trig[0:DH, sl], op=ALU.mult)
    nc.vector.tensor_tensor(tb[:], qps[DH:128, :], trig[DH:128, sl], op=ALU.mult)
    nc.vector.tensor_tensor(tcc[:], qps[DH:128, :], trig[0:DH, sl], op=ALU.mult)
    nc.vector.tensor_tensor(td[:], qps[0:DH, :], trig[DH:128, sl], op=ALU.mult)
    # PSUM bank free from here on
    nc.vector.tensor_add(rot[0:DH, :], ta[:], tb[:])
    nc.vector.tensor_sub(rot[DH:128, :], tcc[:], td[:])
    return rot


def _host_prep(inputs):
    x = np.asarray(inputs["x"], dtype=np.float32)
    Wq = np.asarray(inputs["Wq"], dtype=np.float32)
    Wk = np.asarray(inputs["Wk"], dtype=np.float32)
    Wv = np.asarray(inputs["Wv"], dtype=np.float32)
    Wo = np.asarray(inputs["Wo"], dtype=np.float32)
    w_omega = np.asarray(inputs["w_omega"], dtype=np.float32)
    b_omega = np.asarray(inputs["b_omega"], dtype=np.float32)
    log_freq = np.asarray(inputs["log_freq"], dtype=np.float32)
    q_gamma = np.asarray(inputs["q_gamma"], dtype=np.float32)
    k_gamma = np.asarray(inputs["k_gamma"], dtype=np.float32)

    womg = _round_f32r(w_omega.reshape(NCT, 128).T)  # [p, i] = w_omega[i*128+p]
    b16 = (b_omega / 16.0).reshape(1, 1).astype(np.float32)
    logf = log_freq.reshape(DH, 1)
    gq = q_gamma.reshape(128, 1)
    gk = k_gamma.reshape(128, 1)
    p = np.arange(128)[:, None]
    c = np.arange(512)[None, :]
    masks = np.concatenate(
        [((p + r * 128) <= c).astype(np.float32) for r in range(4)], axis=1
    ).astype(ml_dtypes.bfloat16)
    onesA = np.ones((128, 1), dtype=np.float32)
    onesB = np.ones((1, 128), dtype=np.float32)
    ones64 = np.ones((1, DH), dtype=np.float32)
    oneh31 = np.zeros((128, 31), dtype=np.float32)
    oneh31[:, 15] = 1.0

    in_maps = []
    for core in range(8):
        b, g = core // 2, core % 2
        in_maps.append({
            "xt": _round_f32r(x[b].T),
            "wq": _round_f32r(Wq[g * GD:(g + 1) * GD, :].T),
            "wk": _round_f32r(Wk[g * GD:(g + 1) * GD, :].T),
            "wv": _round_f32r(Wv[g * GD:(g + 1) * GD, :].T),
            "wo": _round_f32r(Wo[:, g * GD:(g + 1) * GD].T),
            "womg": womg, "b16": b16, "logf": logf, "gq": gq, "gk": gk,
            "masks": masks, "onesA": onesA, "onesB": onesB, "ones64": ones64,
            "oneh31": oneh31,
        })
    return in_maps


def kernel(**inputs) -> np.ndarray:
    if "nc" not in _CACHE:
        _CACHE["nc"] = _build()
    nc = _CACHE["nc"]
    in_maps = _host_prep(inputs)
    res = run_bass_kernel_spmd(nc, in_maps, core_ids=list(range(8)))
    out = np.empty((B, T, C), dtype=np.float32)
    for b in range(B):
        out[b] = res.results[2 * b]["out"] + res.results[2 * b + 1]["out"]
    return out
